# revision 1
# baseline (speedup 1.0000x reference)
"""Block-diagonal cross-attention + MLP for trn2, 8-core data-parallel.

v2: 16 graphs/core padded to GCAP=128 rows/side, processed in PAIRS:
  - scores psum tile [128,512] = {SS_a, ST_a, SS_b, ST_b}; one exp ACT op
  - O psum [128,260] = 4x [nodes, 64 V-cols | rsum]; V matmul lhsT = E slice
  - normalize per-partition (recip of the rsum col), PE-transpose into packed
    psum ([0:64)=srcT, [64:128)=tarT via tile_position=(0,64)), evict fused
    with +x residual (packed xT) into eT_packed [128, 2048]
  - MLP on eT_packed with block-diag weights [128,128]: both sides at once
Output: one [128,2048] tensor per core; host unpads/reassembles.
"""

from contextlib import ExitStack

import numpy as np

N_NODES = 8192
D = 64
G = 128
N_CORES = 8
GPC = G // N_CORES          # graphs per core = 16
GCAP = 128                  # padded nodes per graph per side
ROWS = GPC * GCAP           # 2048 padded rows per core
VW = D + 1                  # v-rows width incl. mask column
NPAIR = GPC // 2

_PROGRAM_CACHE = {}


def _build_program(stop_after=None):
    import concourse.bass as bass
    import concourse.tile as tile
    from concourse import bacc, mybir

    fp32 = mybir.dt.float32
    nc = bacc.Bacc("TRN2", target_bir_lowering=False, debug=False)

    xsT = nc.declare_dram_parameter("xsT", [D, ROWS], fp32, isOutput=False)
    xtT = nc.declare_dram_parameter("xtT", [D, ROWS], fp32, isOutput=False)
    vs = nc.declare_dram_parameter("vs", [GCAP, GPC * VW], fp32, isOutput=False)
    vt = nc.declare_dram_parameter("vt", [GCAP, GPC * VW], fp32, isOutput=False)
    w1bd = nc.declare_dram_parameter("w1bd", [2 * D, 2 * D], fp32, isOutput=False)
    b1bd = nc.declare_dram_parameter("b1bd", [2 * D, 1], fp32, isOutput=False)
    w2bd = nc.declare_dram_parameter("w2bd", [2 * D, 2 * D], fp32, isOutput=False)
    b2bd = nc.declare_dram_parameter("b2bd", [2 * D, 1], fp32, isOutput=False)
    ident = nc.declare_dram_parameter("ident", [GCAP, GCAP], fp32, isOutput=False)
    outp = nc.declare_dram_parameter("outp", [2 * D, ROWS], fp32, isOutput=True)

    AF = mybir.ActivationFunctionType
    ALU = mybir.AluOpType

    with tile.TileContext(nc) as tc, ExitStack() as ctx:
        singles = ctx.enter_context(tc.tile_pool(name="singles", bufs=1))
        epool = ctx.enter_context(tc.tile_pool(name="epool", bufs=3))
        work = ctx.enter_context(tc.tile_pool(name="work", bufs=3))

        sb_xsT = singles.tile([D, ROWS], fp32, tag="xsT")
        sb_xtT = singles.tile([D, ROWS], fp32, tag="xtT")
        sb_xp = singles.tile([2 * D, ROWS], fp32, tag="xp")
        sb_vs = singles.tile([GCAP, GPC * VW], fp32, tag="vs")
        sb_vt = singles.tile([GCAP, GPC * VW], fp32, tag="vt")
        sb_w1 = singles.tile([2 * D, 2 * D], fp32, tag="w1")
        sb_b1 = singles.tile([2 * D, 1], fp32, tag="b1")
        sb_w2 = singles.tile([2 * D, 2 * D], fp32, tag="w2")
        sb_b2 = singles.tile([2 * D, 1], fp32, tag="b2")
        sb_id = singles.tile([GCAP, GCAP], fp32, tag="ident")
        sb_eT = singles.tile([2 * D, ROWS], fp32, tag="eT")
        sb_h = singles.tile([2 * D, ROWS], fp32, tag="h")
        sb_out = singles.tile([2 * D, ROWS], fp32, tag="out")

        nc.sync.dma_start(out=sb_xsT, in_=xsT[:, :])
        nc.sync.dma_start(out=sb_xtT, in_=xtT[:, :])
        nc.sync.dma_start(out=sb_vs, in_=vs[:, :])
        nc.sync.dma_start(out=sb_vt, in_=vt[:, :])
        nc.sync.dma_start(out=sb_w1, in_=w1bd[:, :])
        nc.sync.dma_start(out=sb_b1, in_=b1bd[:, :])
        nc.sync.dma_start(out=sb_w2, in_=w2bd[:, :])
        nc.sync.dma_start(out=sb_b2, in_=b2bd[:, :])
        nc.sync.dma_start(out=sb_id, in_=ident[:, :])
        # packed xT for the fused evict+residual (on-chip copies)
        nc.sync.dma_start(out=sb_xp[0:D, :], in_=sb_xsT)
        nc.sync.dma_start(out=sb_xp[D:2 * D, :], in_=sb_xtT)

        with tc.tile_pool(name="ps_sc", bufs=3, space="PSUM") as ps_sc, \
             tc.tile_pool(name="ps_ot", bufs=3, space="PSUM") as ps_ot, \
             tc.tile_pool(name="ps_m", bufs=2, space="PSUM") as ps_m:
            for k in range(NPAIR):
                a, b = 2 * k, 2 * k + 1
                ca, cb = a * GCAP, b * GCAP
                va, vb = a * VW, b * VW
                xs_a = sb_xsT[:, ca:ca + GCAP]
                xt_a = sb_xtT[:, ca:ca + GCAP]
                xs_b = sb_xsT[:, cb:cb + GCAP]
                xt_b = sb_xtT[:, cb:cb + GCAP]

                if stop_after == "dma":
                    continue
                sc = ps_sc.tile([GCAP, 4 * GCAP], fp32, tag="sc")
                nc.tensor.matmul(sc[:, 0:128], xs_a, xt_a, start=True, stop=True)
                nc.tensor.matmul(sc[:, 128:256], xt_a, xs_a, start=True, stop=True)
                nc.tensor.matmul(sc[:, 256:384], xs_b, xt_b, start=True, stop=True)
                nc.tensor.matmul(sc[:, 384:512], xt_b, xs_b, start=True, stop=True)

                et = epool.tile([GCAP, 4 * GCAP], fp32, tag="E")
                nc.scalar.activation(out=et, in_=sc, func=AF.Exp)
                if stop_after == "scores":
                    nc.sync.dma_start(out=outp[:, ca:ca + GCAP], in_=et[:, 0:GCAP])
                    continue

                o = ps_ot.tile([GCAP, 4 * VW], fp32, tag="ot")
                nc.tensor.matmul(o[:, 0:VW], et[:, 128:256],
                                 sb_vt[:, va:va + VW], start=True, stop=True)
                nc.tensor.matmul(o[:, VW:2 * VW], et[:, 0:128],
                                 sb_vs[:, va:va + VW], start=True, stop=True)
                nc.tensor.matmul(o[:, 2 * VW:3 * VW], et[:, 384:512],
                                 sb_vt[:, vb:vb + VW], start=True, stop=True)
                nc.tensor.matmul(o[:, 3 * VW:4 * VW], et[:, 256:384],
                                 sb_vs[:, vb:vb + VW], start=True, stop=True)

                o3 = o.rearrange("p (q w) -> p q w", q=4)
                rc = work.tile([GCAP, 4], fp32, tag="rc")
                nc.vector.reciprocal(out=rc, in_=o3[:, :, D:D + 1])

                er = work.tile([GCAP, 4 * D], fp32, tag="er")
                for j in range(4):
                    if j % 2 == 0:
                        nc.scalar.mul(er[:, j * D:(j + 1) * D],
                                      o[:, j * VW:j * VW + D], rc[:, j:j + 1])
                    else:
                        nc.vector.tensor_scalar_mul(
                            er[:, j * D:(j + 1) * D],
                            o[:, j * VW:j * VW + D], rc[:, j:j + 1])

                if stop_after == "er":
                    nc.sync.dma_start(out=outp[:, ca:ca + 2 * GCAP], in_=er)
                    continue

                tp = ps_ot.tile([2 * D, 2 * GCAP], fp32, tag="ot")
                # er block layout [src|tar] transposes straight into the
                # packed [srcT; tarT] partition layout
                nc.tensor.transpose(tp[:, 0:GCAP], er[:, 0:2 * D], sb_id)
                nc.tensor.transpose(tp[:, GCAP:2 * GCAP], er[:, 2 * D:4 * D], sb_id)
                # fused evict + residual
                nc.vector.tensor_add(sb_eT[:, ca:ca + 2 * GCAP], tp,
                                     sb_xp[:, ca:ca + 2 * GCAP])

            # ---- packed MLP over [128, 2048] in chunks of 512 ----
            CH = 512
            if stop_after is None:
                for c in range(0, ROWS, CH):
                    hp = ps_m.tile([2 * D, CH], fp32, tag="m")
                    nc.tensor.matmul(hp, sb_w1, sb_eT[:, c:c + CH],
                                     start=True, stop=True)
                    nc.vector.tensor_scalar(
                        out=sb_h[:, c:c + CH], in0=hp, scalar1=sb_b1,
                        scalar2=0.0, op0=ALU.add, op1=ALU.max)
                for c in range(0, ROWS, CH):
                    op2 = ps_m.tile([2 * D, CH], fp32, tag="m")
                    nc.tensor.matmul(op2, sb_w2, sb_h[:, c:c + CH],
                                     start=True, stop=False)
                    # residual folded into psum via identity matmul
                    nc.tensor.matmul(op2, sb_id, sb_eT[:, c:c + CH],
                                     start=False, stop=True)
                    nc.scalar.activation(out=sb_out[:, c:c + CH], in_=op2,
                                         func=AF.Identity, bias=sb_b2, scale=1.0)
                nc.sync.dma_start(out=outp[:, :], in_=sb_out)
            elif stop_after == "attn":
                nc.sync.dma_start(out=outp[:, :], in_=sb_eT)
            elif stop_after == "dma":
                nc.sync.dma_start(out=outp[0:D, :], in_=sb_xsT)

    nc.compile()
    return nc


def _shard_inputs(x_src, batch_src, x_tar, batch_tar, w1, b1, w2, b2):
    """Build per-core padded DRAM images. Returns (in_maps, meta)."""
    bs = np.asarray(batch_src).astype(np.int64)
    bt = np.asarray(batch_tar).astype(np.int64)
    xs = np.asarray(x_src, dtype=np.float32)
    xt = np.asarray(x_tar, dtype=np.float32)

    bnd_s = np.searchsorted(bs, np.arange(G + 1))
    bnd_t = np.searchsorted(bt, np.arange(G + 1))
    cnt_s = np.diff(bnd_s)
    cnt_t = np.diff(bnd_t)
    if cnt_s.max(initial=0) > GCAP or cnt_t.max(initial=0) > GCAP:
        return None, (bnd_s, bnd_t, cnt_s, cnt_t)

    w1a = np.asarray(w1, dtype=np.float32)
    w2a = np.asarray(w2, dtype=np.float32)
    b1a = np.asarray(b1, dtype=np.float32).reshape(D)
    b2a = np.asarray(b2, dtype=np.float32).reshape(D)
    w1bd = np.zeros((2 * D, 2 * D), dtype=np.float32)
    w2bd = np.zeros((2 * D, 2 * D), dtype=np.float32)
    w1bd[:D, :D] = w1a; w1bd[D:, D:] = w1a
    w2bd[:D, :D] = w2a; w2bd[D:, D:] = w2a
    b1bd = np.concatenate([b1a, b1a]).reshape(2 * D, 1)
    b2bd = np.concatenate([b2a, b2a]).reshape(2 * D, 1)
    ident = np.eye(GCAP, dtype=np.float32)

    in_maps = []
    for c in range(N_CORES):
        xs_pad = np.zeros((GPC, GCAP, D), dtype=np.float32)
        xt_pad = np.zeros((GPC, GCAP, D), dtype=np.float32)
        ms = np.zeros((GPC, GCAP), dtype=np.float32)
        mt = np.zeros((GPC, GCAP), dtype=np.float32)
        for i in range(GPC):
            g = c * GPC + i
            ns, nt = cnt_s[g], cnt_t[g]
            xs_pad[i, :ns] = xs[bnd_s[g]:bnd_s[g + 1]]
            xt_pad[i, :nt] = xt[bnd_t[g]:bnd_t[g + 1]]
            ms[i, :ns] = 1.0
            mt[i, :nt] = 1.0
        xsT_img = np.ascontiguousarray(xs_pad.reshape(ROWS, D).T)
        xtT_img = np.ascontiguousarray(xt_pad.reshape(ROWS, D).T)
        vs_img = np.concatenate(
            [xs_pad.transpose(1, 0, 2), ms.T[:, :, None]], axis=2)
        vt_img = np.concatenate(
            [xt_pad.transpose(1, 0, 2), mt.T[:, :, None]], axis=2)
        in_maps.append({
            "xsT": xsT_img,
            "xtT": xtT_img,
            "vs": np.ascontiguousarray(vs_img.reshape(GCAP, GPC * VW)),
            "vt": np.ascontiguousarray(vt_img.reshape(GCAP, GPC * VW)),
            "w1bd": w1bd, "b1bd": b1bd, "w2bd": w2bd, "b2bd": b2bd,
            "ident": ident,
        })
    return in_maps, (bnd_s, bnd_t, cnt_s, cnt_t)


def _numpy_fallback(x_src, batch_src, x_tar, batch_tar, w1, b1, w2, b2):
    bs = np.asarray(batch_src); bt = np.asarray(batch_tar)
    xs = np.asarray(x_src, dtype=np.float64); xt = np.asarray(x_tar, dtype=np.float64)
    mask = bs[:, None] == bt[None, :]

    def attend(q, kv, m):
        s = np.where(m, q @ kv.T, -1.0e9)
        s = s - s.max(axis=1, keepdims=True)
        e = np.exp(s)
        a = e / e.sum(axis=1, keepdims=True)
        out = a @ kv + q
        return np.where(m.any(axis=1, keepdims=True), out, 0.0)

    def mlp(x):
        return np.maximum(x @ w1 + b1, 0.0) @ w2 + b2 + x

    es = mlp(attend(xs, xt, mask))
    et = mlp(attend(xt, xs, mask.T))
    return et.astype(np.float32), es.astype(np.float32)


def kernel(x_src, batch_src, x_tar, batch_tar, w1, b1, w2, b2):
    in_maps, meta = _shard_inputs(
        x_src, batch_src, x_tar, batch_tar, w1, b1, w2, b2)
    if in_maps is None:  # a graph overflowed GCAP; never happens for spec data
        return _numpy_fallback(
            x_src, batch_src, x_tar, batch_tar, w1, b1, w2, b2)
    bnd_s, bnd_t, cnt_s, cnt_t = meta

    import os
    from concourse import bass_utils
    if "nc" not in _PROGRAM_CACHE:
        _PROGRAM_CACHE["nc"] = _build_program()
    nc = _PROGRAM_CACHE["nc"]
    trace = bool(os.environ.get("KERNEL_TRACE"))
    res = bass_utils.run_bass_kernel_spmd(
        nc, in_maps, core_ids=list(range(N_CORES)), trace=trace)
    _PROGRAM_CACHE["last_result"] = res

    embed_src = np.zeros((N_NODES, D), dtype=np.float32)
    embed_tar = np.zeros((N_NODES, D), dtype=np.float32)
    for c in range(N_CORES):
        op = np.asarray(res.results[c]["outp"])  # [128, ROWS]
        o_s = op[0:D]
        o_t = op[D:2 * D]
        for i in range(GPC):
            g = c * GPC + i
            ns, nt = cnt_s[g], cnt_t[g]
            if nt > 0:  # src rows valid only if tar side nonempty
                embed_src[bnd_s[g]:bnd_s[g] + ns] = o_s[:, i * GCAP:i * GCAP + ns].T
            if ns > 0:
                embed_tar[bnd_t[g]:bnd_t[g] + nt] = o_t[:, i * GCAP:i * GCAP + nt].T
    return embed_tar, embed_src



# revision 2
# speedup vs baseline: 1.4307x; 1.4307x over previous
"""Block-diagonal cross-attention + MLP for trn2, 8-core data-parallel.

v3: bf16 matmuls (single-pass, FWL) + per-pair streamed input DMA +
GCAP auto-sized to the actual max graph size (rounded up to 32) +
chunked output DMA overlapped with the MLP.

Per core: GPC=16 graphs padded to GCAP rows/side, processed in PAIRS:
  - scores psum [GCAP, 4*GCAP] = {SS_a, ST_a, SS_b, ST_b}; one exp ACT -> bf16
  - O psum [GCAP, 4*65] = 4x [64 V-cols | rsum]; lhsT = E slice, rhs = v+mask
  - normalize per-partition (recip of rsum col) -> er fp32
  - PE-transpose er (both graphs) into tp psum [128, 2*GCAP], evict fused
    with +x residual (host-packed xT image) into eT bf16 [128, ROWS]
  - MLP on eT with block-diag weights [128,128] bf16, residual folded in
    via identity matmul; per-chunk fp32 output DMA.
Output: one [128, ROWS] fp32 tensor per core; host unpads/reassembles.
"""

from contextlib import ExitStack

import numpy as np
import ml_dtypes

BF16 = ml_dtypes.bfloat16

N_NODES = 8192
D = 64
G = 128
N_CORES = 8
GPC = G // N_CORES          # graphs per core = 16
VW = D + 1                  # v-rows width incl. mask column
NPAIR = GPC // 2

_PROGRAM_CACHE = {}


def _build_program(gcap):
    import concourse.bass as bass
    import concourse.tile as tile
    from concourse import bacc, mybir

    fp32 = mybir.dt.float32
    bf16 = mybir.dt.bfloat16
    rows = GPC * gcap
    nc = bacc.Bacc("TRN2", target_bir_lowering=False, debug=False)

    # pair-contiguous DRAM images
    xp = nc.declare_dram_parameter("xp", [NPAIR * 128, 2 * gcap], bf16, isOutput=False)
    xs = nc.declare_dram_parameter("xs", [NPAIR * D, 2 * gcap], bf16, isOutput=False)
    xt = nc.declare_dram_parameter("xt", [NPAIR * D, 2 * gcap], bf16, isOutput=False)
    vs = nc.declare_dram_parameter("vs", [NPAIR * gcap, 2 * VW], bf16, isOutput=False)
    vt = nc.declare_dram_parameter("vt", [NPAIR * gcap, 2 * VW], bf16, isOutput=False)
    w1bd = nc.declare_dram_parameter("w1bd", [2 * D, 2 * D], bf16, isOutput=False)
    b1bd = nc.declare_dram_parameter("b1bd", [2 * D, 1], fp32, isOutput=False)
    w2bd = nc.declare_dram_parameter("w2bd", [2 * D, 2 * D], bf16, isOutput=False)
    b2bd = nc.declare_dram_parameter("b2bd", [2 * D, 1], fp32, isOutput=False)
    identf = nc.declare_dram_parameter("identf", [gcap, gcap], fp32, isOutput=False)
    identb = nc.declare_dram_parameter("identb", [2 * D, 2 * D], bf16, isOutput=False)
    outp = nc.declare_dram_parameter("outp", [2 * D, rows], fp32, isOutput=True)

    AF = mybir.ActivationFunctionType
    ALU = mybir.AluOpType

    with tile.TileContext(nc) as tc, ExitStack() as ctx:
        singles = ctx.enter_context(tc.tile_pool(name="singles", bufs=1))
        epool = ctx.enter_context(tc.tile_pool(name="epool", bufs=3))
        work = ctx.enter_context(tc.tile_pool(name="work", bufs=3))
        opool = ctx.enter_context(tc.tile_pool(name="opool", bufs=2))

        sb_xp = singles.tile([128, rows], bf16, tag="xp")
        sb_xs = singles.tile([D, rows], bf16, tag="xs")
        sb_xt = singles.tile([D, rows], bf16, tag="xt")
        sb_vs = singles.tile([gcap, GPC * VW], bf16, tag="vs")
        sb_vt = singles.tile([gcap, GPC * VW], bf16, tag="vt")
        sb_w1 = singles.tile([2 * D, 2 * D], bf16, tag="w1")
        sb_b1 = singles.tile([2 * D, 1], fp32, tag="b1")
        sb_w2 = singles.tile([2 * D, 2 * D], bf16, tag="w2")
        sb_b2 = singles.tile([2 * D, 1], fp32, tag="b2")
        sb_idf = singles.tile([gcap, gcap], fp32, tag="identf")
        sb_idb = singles.tile([2 * D, 2 * D], bf16, tag="identb")
        sb_eT = singles.tile([2 * D, rows], bf16, tag="eT")
        sb_h = singles.tile([2 * D, rows], bf16, tag="h")

        # constants first (small)
        nc.sync.dma_start(out=sb_w1, in_=w1bd[:, :])
        nc.sync.dma_start(out=sb_b1, in_=b1bd[:, :])
        nc.sync.dma_start(out=sb_w2, in_=w2bd[:, :])
        nc.sync.dma_start(out=sb_b2, in_=b2bd[:, :])
        nc.sync.dma_start(out=sb_idf, in_=identf[:, :])
        nc.sync.dma_start(out=sb_idb, in_=identb[:, :])
        # per-pair streamed inputs
        xp3 = xp.rearrange("(k p) c -> k p c", p=128)
        xs3 = xs.rearrange("(k p) c -> k p c", p=D)
        xt3 = xt.rearrange("(k p) c -> k p c", p=D)
        vs3 = vs.rearrange("(k p) c -> k p c", p=gcap)
        vt3 = vt.rearrange("(k p) c -> k p c", p=gcap)
        for k in range(NPAIR):
            c2 = 2 * k * gcap
            nc.sync.dma_start(out=sb_xs[:, c2:c2 + 2 * gcap], in_=xs3[k])
            nc.sync.dma_start(out=sb_xt[:, c2:c2 + 2 * gcap], in_=xt3[k])
            nc.sync.dma_start(out=sb_vs[:, 2 * k * VW:(2 * k + 2) * VW], in_=vs3[k])
            nc.sync.dma_start(out=sb_vt[:, 2 * k * VW:(2 * k + 2) * VW], in_=vt3[k])
            nc.sync.dma_start(out=sb_xp[:, c2:c2 + 2 * gcap], in_=xp3[k])

        with tc.tile_pool(name="ps_sc", bufs=3, space="PSUM") as ps_sc, \
             tc.tile_pool(name="ps_ot", bufs=3, space="PSUM") as ps_ot, \
             tc.tile_pool(name="ps_m", bufs=2, space="PSUM") as ps_m:
            for k in range(NPAIR):
                ca = 2 * k * gcap          # col base of graph a in x/eT space
                cb = ca + gcap
                va, vb = 2 * k * VW, (2 * k + 1) * VW
                xs_a = sb_xs[:, ca:ca + gcap]
                xt_a = sb_xt[:, ca:ca + gcap]
                xs_b = sb_xs[:, cb:cb + gcap]
                xt_b = sb_xt[:, cb:cb + gcap]

                sc = ps_sc.tile([gcap, 4 * gcap], fp32, tag="sc")
                nc.tensor.matmul(sc[:, 0:gcap], xs_a, xt_a, start=True, stop=True)
                nc.tensor.matmul(sc[:, gcap:2 * gcap], xt_a, xs_a, start=True, stop=True)
                nc.tensor.matmul(sc[:, 2 * gcap:3 * gcap], xs_b, xt_b, start=True, stop=True)
                nc.tensor.matmul(sc[:, 3 * gcap:4 * gcap], xt_b, xs_b, start=True, stop=True)

                et = epool.tile([gcap, 4 * gcap], bf16, tag="E")
                nc.scalar.activation(out=et, in_=sc, func=AF.Exp)

                o = ps_ot.tile([gcap, 4 * VW], fp32, tag="ot")
                nc.tensor.matmul(o[:, 0:VW], et[:, gcap:2 * gcap],
                                 sb_vt[:, va:va + VW], start=True, stop=True)
                nc.tensor.matmul(o[:, VW:2 * VW], et[:, 0:gcap],
                                 sb_vs[:, va:va + VW], start=True, stop=True)
                nc.tensor.matmul(o[:, 2 * VW:3 * VW], et[:, 3 * gcap:4 * gcap],
                                 sb_vt[:, vb:vb + VW], start=True, stop=True)
                nc.tensor.matmul(o[:, 3 * VW:4 * VW], et[:, 2 * gcap:3 * gcap],
                                 sb_vs[:, vb:vb + VW], start=True, stop=True)

                o3 = o.rearrange("p (q w) -> p q w", q=4)
                rc = work.tile([gcap, 4], fp32, tag="rc")
                nc.vector.reciprocal(out=rc, in_=o3[:, :, D:D + 1])

                er = work.tile([gcap, 4 * D], fp32, tag="er")
                for j in range(4):
                    if j % 2 == 0:
                        nc.scalar.mul(er[:, j * D:(j + 1) * D],
                                      o[:, j * VW:j * VW + D], rc[:, j:j + 1])
                    else:
                        nc.vector.tensor_scalar_mul(
                            er[:, j * D:(j + 1) * D],
                            o[:, j * VW:j * VW + D], rc[:, j:j + 1])

                tp = ps_ot.tile([2 * D, 2 * gcap], fp32, tag="ot")
                # er block layout [src|tar] transposes straight into the
                # packed [srcT; tarT] partition layout
                nc.tensor.transpose(tp[:, 0:gcap], er[:, 0:2 * D], sb_idf)
                nc.tensor.transpose(tp[:, gcap:2 * gcap], er[:, 2 * D:4 * D], sb_idf)
                # fused evict + residual (packed host-sent xT image)
                nc.vector.tensor_add(sb_eT[:, ca:ca + 2 * gcap], tp,
                                     sb_xp[:, ca:ca + 2 * gcap])

            # ---- packed MLP over [128, rows] in chunks ----
            CH = 512
            nch = (rows + CH - 1) // CH
            for ci in range(nch):
                c = ci * CH
                w = min(CH, rows - c)
                hp = ps_m.tile([2 * D, CH], fp32, tag="m")
                nc.tensor.matmul(hp[:, 0:w], sb_w1, sb_eT[:, c:c + w],
                                 start=True, stop=True)
                nc.vector.tensor_scalar(
                    out=sb_h[:, c:c + w], in0=hp[:, 0:w], scalar1=sb_b1,
                    scalar2=0.0, op0=ALU.add, op1=ALU.max)
            for ci in range(nch):
                c = ci * CH
                w = min(CH, rows - c)
                op2 = ps_m.tile([2 * D, CH], fp32, tag="m")
                nc.tensor.matmul(op2[:, 0:w], sb_w2, sb_h[:, c:c + w],
                                 start=True, stop=False)
                # residual folded into psum via identity matmul
                nc.tensor.matmul(op2[:, 0:w], sb_idb, sb_eT[:, c:c + w],
                                 start=False, stop=True)
                ot = opool.tile([2 * D, CH], fp32, tag="out")
                nc.scalar.activation(out=ot[:, 0:w], in_=op2[:, 0:w],
                                     func=AF.Identity, bias=sb_b2, scale=1.0)
                nc.sync.dma_start(out=outp[:, c:c + w], in_=ot[:, 0:w])

    nc.compile()
    return nc


def _shard_inputs(x_src, batch_src, x_tar, batch_tar, w1, b1, w2, b2, gcap):
    """Build per-core padded bf16 DRAM images. Returns (in_maps, meta)."""
    bs = np.asarray(batch_src).astype(np.int64)
    bt = np.asarray(batch_tar).astype(np.int64)
    xsf = np.asarray(x_src, dtype=np.float32)
    xtf = np.asarray(x_tar, dtype=np.float32)
    rows = GPC * gcap

    bnd_s = np.searchsorted(bs, np.arange(G + 1))
    bnd_t = np.searchsorted(bt, np.arange(G + 1))

    n_s = np.arange(N_NODES)
    n_t = np.arange(N_NODES)
    ws_ = n_s - bnd_s[bs]              # index within graph (src)
    wt_ = n_t - bnd_t[bt]
    cs_ = bs // GPC                    # core id
    ct_ = bt // GPC
    is_ = bs % GPC                     # graph slot within core
    it_ = bt % GPC
    col_s = is_ * gcap + ws_           # column in per-core x/eT space
    col_t = it_ * gcap + wt_

    xsb = xsf.astype(BF16)
    xtb = xtf.astype(BF16)

    # packed xT image [8][128][rows]: rows 0:64 srcT, 64:128 tarT
    xp_all = np.zeros((N_CORES, 128, rows), dtype=BF16)
    xp_all[cs_, :D, col_s] = xsb
    xp_all[ct_, D:, col_t] = xtb
    xs_all = np.ascontiguousarray(xp_all[:, :D, :])
    xt_all = np.ascontiguousarray(xp_all[:, D:, :])

    # value images [8][gcap][GPC*VW] with mask col
    vs_all = np.zeros((N_CORES, gcap, GPC, VW), dtype=BF16)
    vt_all = np.zeros((N_CORES, gcap, GPC, VW), dtype=BF16)
    vs_all[cs_, ws_, is_, :D] = xsb
    vs_all[cs_, ws_, is_, D] = 1.0
    vt_all[ct_, wt_, it_, :D] = xtb
    vt_all[ct_, wt_, it_, D] = 1.0

    def pairblock(a, p):
        # [8, p, NPAIR, c2] -> [8, NPAIR*p, c2] pair-contiguous
        n, _, tot = a.shape
        c2 = tot // NPAIR
        return np.ascontiguousarray(
            a.reshape(n, p, NPAIR, c2).transpose(0, 2, 1, 3).reshape(n, NPAIR * p, c2))

    w1a = np.asarray(w1, dtype=np.float32)
    w2a = np.asarray(w2, dtype=np.float32)
    b1a = np.asarray(b1, dtype=np.float32).reshape(D)
    b2a = np.asarray(b2, dtype=np.float32).reshape(D)
    w1bd = np.zeros((2 * D, 2 * D), dtype=np.float32)
    w2bd = np.zeros((2 * D, 2 * D), dtype=np.float32)
    w1bd[:D, :D] = w1a; w1bd[D:, D:] = w1a
    w2bd[:D, :D] = w2a; w2bd[D:, D:] = w2a
    b1bd = np.concatenate([b1a, b1a]).reshape(2 * D, 1)
    b2bd = np.concatenate([b2a, b2a]).reshape(2 * D, 1)
    identf = np.eye(gcap, dtype=np.float32)
    identb = np.eye(2 * D, dtype=BF16)

    in_maps = []
    for c in range(N_CORES):
        in_maps.append({
            "xp": pairblock(xp_all[c][None], 128)[0],
            "xs": pairblock(xs_all[c][None], D)[0],
            "xt": pairblock(xt_all[c][None], D)[0],
            "vs": pairblock(vs_all[c].reshape(1, gcap, GPC * VW), gcap)[0],
            "vt": pairblock(vt_all[c].reshape(1, gcap, GPC * VW), gcap)[0],
            "w1bd": w1bd.astype(BF16), "b1bd": b1bd,
            "w2bd": w2bd.astype(BF16), "b2bd": b2bd,
            "identf": identf, "identb": identb,
        })
    meta = (bnd_s, bnd_t, cs_, ct_, col_s, col_t)
    return in_maps, meta


def _numpy_fallback(x_src, batch_src, x_tar, batch_tar, w1, b1, w2, b2):
    bs = np.asarray(batch_src); bt = np.asarray(batch_tar)
    xs = np.asarray(x_src, dtype=np.float64); xt = np.asarray(x_tar, dtype=np.float64)
    mask = bs[:, None] == bt[None, :]

    def attend(q, kv, m):
        s = np.where(m, q @ kv.T, -1.0e9)
        s = s - s.max(axis=1, keepdims=True)
        e = np.exp(s)
        a = e / e.sum(axis=1, keepdims=True)
        out = a @ kv + q
        return np.where(m.any(axis=1, keepdims=True), out, 0.0)

    def mlp(x):
        return np.maximum(x @ w1 + b1, 0.0) @ w2 + b2 + x

    es = mlp(attend(xs, xt, mask))
    et = mlp(attend(xt, xs, mask.T))
    return et.astype(np.float32), es.astype(np.float32)


def kernel(x_src, batch_src, x_tar, batch_tar, w1, b1, w2, b2):
    bs = np.asarray(batch_src).astype(np.int64)
    bt = np.asarray(batch_tar).astype(np.int64)
    cnt_s = np.bincount(bs, minlength=G)
    cnt_t = np.bincount(bt, minlength=G)
    maxc = int(max(cnt_s.max(initial=0), cnt_t.max(initial=0)))
    if maxc > 128 or bs.min(initial=0) < 0 or bs.max(initial=0) >= G \
            or bt.min(initial=0) < 0 or bt.max(initial=0) >= G:
        return _numpy_fallback(
            x_src, batch_src, x_tar, batch_tar, w1, b1, w2, b2)
    gcap = max(32, -(-maxc // 32) * 32)   # round up to multiple of 32

    in_maps, meta = _shard_inputs(
        x_src, batch_src, x_tar, batch_tar, w1, b1, w2, b2, gcap)
    bnd_s, bnd_t, cs_, ct_, col_s, col_t = meta

    import os
    from concourse import bass_utils
    if gcap not in _PROGRAM_CACHE:
        _PROGRAM_CACHE[gcap] = _build_program(gcap)
    nc = _PROGRAM_CACHE[gcap]
    trace = bool(os.environ.get("KERNEL_TRACE"))
    res = bass_utils.run_bass_kernel_spmd(
        nc, in_maps, core_ids=list(range(N_CORES)), trace=trace)
    _PROGRAM_CACHE["last_result"] = res

    outs = np.stack([np.asarray(res.results[c]["outp"]) for c in range(N_CORES)])
    # gather: embed_src from rows 0:64 at (core, col_s); tar from rows 64:128
    embed_src = np.ascontiguousarray(outs[cs_, :D, col_s])
    embed_tar = np.ascontiguousarray(outs[ct_, D:, col_t])
    # rows whose graph has no counterpart stay zero
    embed_src[cnt_t[bs] == 0] = 0.0
    embed_tar[cnt_s[bt] == 0] = 0.0
    return embed_tar, embed_src


# revision 4
# speedup vs baseline: 2.1293x; 1.4882x over previous
"""Block-diagonal cross-attention + MLP for trn2, 8-core data-parallel.

v4: size-sorted graph banding + fp16 score matmuls + superpair batching.

- 128 graphs sorted by size desc into 4 bands of 32; band k gets a shared
  cap[k] = roundup(max size, 8). Each core takes 4 graphs from each band
  (one superpair/SP of 4 graphs per band) -> identical shapes across cores
  (SPMD) with ~30% less padding than a global cap.
- Scores in fp16 (4x the mantissa of bf16 at the same PE speed); E, values,
  weights, eT in bf16; PSUM/normalization in fp32; output fp32.
- Per SP: 8 score MMs into a 2-bank psum [cap, 1024] (4 blocks/bank, never
  crossing a bank), ONE exp ACT (strided) -> et bf16, 8 V MMs into the
  psum slot (values + mask-col rowsum), ONE reciprocal, ONE broadcast-AP
  normalize mul -> er bf16, 4 PE transposes -> tp psum, ONE fused
  evict+residual add -> eT.
- MLP over eT in <=512 chunks; bias1+relu alternates ACT/DVE; residual via
  identity matmul; per-chunk output DMA.
- 7 input DMAs + 3 output DMAs total, split across sync/gpsimd queues.
"""

from contextlib import ExitStack

import numpy as np
import ml_dtypes

BF16 = ml_dtypes.bfloat16

N_NODES = 8192
D = 64
G = 128
N_CORES = 8
GPC = G // N_CORES          # graphs per core = 16
VW = D + 1                  # value width incl. mask column
NBAND = 4                   # bands == superpairs per core
BANDG = G // NBAND          # graphs per band = 32

_PROGRAM_CACHE = {}


def _build_program(caps):
    import concourse.bass as bass
    import concourse.tile as tile
    from concourse import bacc, mybir

    fp32 = mybir.dt.float32
    bf16 = mybir.dt.bfloat16
    fp16 = mybir.dt.float16
    caps = list(caps)
    cap0 = max(caps)
    off = [4 * sum(caps[:k]) for k in range(NBAND)]
    ct = 4 * sum(caps)                  # total node columns per core
    nc = bacc.Bacc("TRN2", target_bir_lowering=False, debug=False)

    xs = nc.declare_dram_parameter("xs", [D, ct], fp16, isOutput=False)
    xt = nc.declare_dram_parameter("xt", [D, ct], fp16, isOutput=False)
    xp = nc.declare_dram_parameter("xp", [2 * D, ct], bf16, isOutput=False)
    vs = nc.declare_dram_parameter("vs", [cap0, GPC * VW], bf16, isOutput=False)
    vt = nc.declare_dram_parameter("vt", [cap0, GPC * VW], bf16, isOutput=False)
    # w1 | w2 | ident packed [128, 384] bf16; b1 | b2 packed [128, 2] fp32
    cwb = nc.declare_dram_parameter("cwb", [2 * D, 6 * D], bf16, isOutput=False)
    cbf = nc.declare_dram_parameter("cbf", [2 * D, 2], fp32, isOutput=False)
    outp = nc.declare_dram_parameter("outp", [2 * D, ct], fp32, isOutput=True)

    AF = mybir.ActivationFunctionType
    ALU = mybir.AluOpType

    with tile.TileContext(nc) as tc, ExitStack() as ctx:
        singles = ctx.enter_context(tc.tile_pool(name="singles", bufs=1))
        epool = ctx.enter_context(tc.tile_pool(name="epool", bufs=3))
        work = ctx.enter_context(tc.tile_pool(name="work", bufs=3))
        opool = ctx.enter_context(tc.tile_pool(name="opool", bufs=2))

        sb_xs = singles.tile([D, ct], fp16, tag="xs")
        sb_xt = singles.tile([D, ct], fp16, tag="xt")
        sb_xp = singles.tile([2 * D, ct], bf16, tag="xp")
        sb_vs = singles.tile([cap0, GPC * VW], bf16, tag="vs")
        sb_vt = singles.tile([cap0, GPC * VW], bf16, tag="vt")
        sb_cw = singles.tile([2 * D, 6 * D], bf16, tag="cwb")
        sb_cb = singles.tile([2 * D, 2], fp32, tag="cbf")
        sb_eT = singles.tile([2 * D, ct], bf16, tag="eT")
        sb_h = singles.tile([2 * D, ct], bf16, tag="h")
        sb_w1 = sb_cw[:, 0:2 * D]
        sb_w2 = sb_cw[:, 2 * D:4 * D]
        sb_id = sb_cw[:, 4 * D:6 * D]
        sb_b1 = sb_cb[:, 0:1]
        sb_b2 = sb_cb[:, 1:2]

        # two independent DMA-issue queues; x first so compute starts early
        nc.sync.dma_start(out=sb_xs, in_=xs[:, :])
        nc.gpsimd.dma_start(out=sb_xt, in_=xt[:, :])
        nc.gpsimd.dma_start(out=sb_cw, in_=cwb[:, :])
        nc.sync.dma_start(out=sb_vs, in_=vs[:, :])
        nc.gpsimd.dma_start(out=sb_vt, in_=vt[:, :])
        nc.sync.dma_start(out=sb_xp, in_=xp[:, :])
        nc.sync.dma_start(out=sb_cb, in_=cbf[:, :])

        with tc.tile_pool(name="ps_a", bufs=3, space="PSUM") as ps_a, \
             tc.tile_pool(name="ps_t", bufs=2, space="PSUM") as ps_t:
            for k in range(NBAND):
                cap = caps[k]
                ob = off[k]

                def scol(j, d):
                    return 512 * (j // 2) + ((j % 2) * 2 + d) * cap

                def ocol(j, d):
                    return 512 * (j // 2) + ((j % 2) * 2 + d) * VW

                sc = ps_a.tile([cap, 1024], fp32, tag="sco")
                for j in range(4):
                    xsj = sb_xs[:, ob + j * cap: ob + (j + 1) * cap]
                    xtj = sb_xt[:, ob + j * cap: ob + (j + 1) * cap]
                    nc.tensor.matmul(sc[:, scol(j, 0):scol(j, 0) + cap],
                                     xsj, xtj, start=True, stop=True)
                    nc.tensor.matmul(sc[:, scol(j, 1):scol(j, 1) + cap],
                                     xtj, xsj, start=True, stop=True)

                et = epool.tile([cap, 8 * cap], bf16, tag="E")
                sc_str = sc.rearrange("p (a c) -> p a c", a=2)[:, :, 0:4 * cap]
                nc.scalar.activation(out=et, in_=sc_str, func=AF.Exp)

                o = ps_a.tile([cap, 1024], fp32, tag="sco")
                for j in range(4):
                    vcol = (4 * k + j) * VW
                    nc.tensor.matmul(
                        o[:, ocol(j, 0):ocol(j, 0) + VW],
                        et[:, (2 * j + 1) * cap:(2 * j + 2) * cap],
                        sb_vt[0:cap, vcol:vcol + VW], start=True, stop=True)
                    nc.tensor.matmul(
                        o[:, ocol(j, 1):ocol(j, 1) + VW],
                        et[:, (2 * j) * cap:(2 * j + 1) * cap],
                        sb_vs[0:cap, vcol:vcol + VW], start=True, stop=True)

                o4 = o.rearrange("p (a c) -> p a c", a=2)[:, :, 0:4 * VW] \
                      .rearrange("p a (b w) -> p a b w", w=VW)
                rc = work.tile([cap, 8], fp32, tag="rc")
                nc.vector.reciprocal(out=rc, in_=o4[:, :, 0:4, D:D + 1])

                er = work.tile([cap, 8 * D], bf16, tag="er")
                rcb = rc.rearrange("p (a b) -> p a b", a=2).to_broadcast(
                    [cap, 2, 4, D])
                nc.vector.tensor_mul(er.rearrange("p (a b w) -> p a b w",
                                                  a=2, w=D),
                                     o4[:, :, 0:4, 0:D], rcb)

                tp = ps_t.tile([2 * D, 4 * cap], bf16, tag="tp")
                for j in range(4):
                    nc.tensor.transpose(tp[:, j * cap:(j + 1) * cap],
                                        er[:, j * 2 * D:(j + 1) * 2 * D],
                                        sb_id[0:cap, 0:cap])
                nc.vector.tensor_add(sb_eT[:, ob:ob + 4 * cap], tp,
                                     sb_xp[:, ob:ob + 4 * cap])

        # ---- packed MLP over [128, ct] in chunks ----
        nch = -(-ct // 512)
        cw = -(-ct // nch)
        cw = -(-cw // 8) * 8
        bounds = [(i * cw, min(ct, (i + 1) * cw)) for i in range(nch)]
        with tc.tile_pool(name="ps_m", bufs=4, space="PSUM") as ps_m:
            for ci, (c, e) in enumerate(bounds):
                w = e - c
                hp = ps_m.tile([2 * D, 512], fp32, tag="m")
                nc.tensor.matmul(hp[:, 0:w], sb_w1, sb_eT[:, c:c + w],
                                 start=True, stop=True)
                if ci % 2 == 0:
                    nc.scalar.activation(out=sb_h[:, c:c + w], in_=hp[:, 0:w],
                                         func=AF.Relu, bias=sb_b1, scale=1.0)
                else:
                    nc.vector.tensor_scalar(
                        out=sb_h[:, c:c + w], in0=hp[:, 0:w], scalar1=sb_b1,
                        scalar2=0.0, op0=ALU.add, op1=ALU.max)
            for ci, (c, e) in enumerate(bounds):
                w = e - c
                op2 = ps_m.tile([2 * D, 512], fp32, tag="m")
                nc.tensor.matmul(op2[:, 0:w], sb_w2, sb_h[:, c:c + w],
                                 start=True, stop=False)
                nc.tensor.matmul(op2[:, 0:w], sb_id, sb_eT[:, c:c + w],
                                 start=False, stop=True)
                ot = opool.tile([2 * D, 512], fp32, tag="out")
                if ci % 2 == 0:
                    nc.scalar.activation(out=ot[:, 0:w], in_=op2[:, 0:w],
                                         func=AF.Identity, bias=sb_b2, scale=1.0)
                else:
                    nc.vector.tensor_scalar_add(ot[:, 0:w], op2[:, 0:w], sb_b2)
                nc.gpsimd.dma_start(out=outp[:, c:c + w], in_=ot[:, 0:w])

    nc.compile()
    return nc


def _plan(cnt_s, cnt_t):
    size = np.maximum(cnt_s, cnt_t)
    order = np.argsort(-size, kind="stable")
    bands = order.reshape(NBAND, BANDG)
    caps = tuple(int(-(-int(size[b].max()) // 8) * 8) for b in bands)
    core_of = np.empty(G, np.int64)
    band_of = np.empty(G, np.int64)
    slot_of = np.empty(G, np.int64)
    for k in range(NBAND):
        for c in range(N_CORES):
            for j in range(4):
                g = bands[k, c * 4 + j]
                core_of[g] = c
                band_of[g] = k
                slot_of[g] = j
    return caps, core_of, band_of, slot_of


def _shard_inputs(x_src, batch_src, x_tar, batch_tar, w1, b1, w2, b2, plan):
    caps, core_of, band_of, slot_of = plan
    bs = np.asarray(batch_src).astype(np.int64)
    bt = np.asarray(batch_tar).astype(np.int64)
    xsf = np.asarray(x_src, dtype=np.float32)
    xtf = np.asarray(x_tar, dtype=np.float32)
    cap0 = max(caps)
    offs = np.array([4 * sum(caps[:k]) for k in range(NBAND)], np.int64)
    capv = np.array(caps, np.int64)
    ct = int(4 * sum(caps))

    bnd_s = np.searchsorted(bs, np.arange(G + 1))
    bnd_t = np.searchsorted(bt, np.arange(G + 1))
    ws_ = np.arange(N_NODES) - bnd_s[bs]
    wt_ = np.arange(N_NODES) - bnd_t[bt]
    cs_, ct_ = core_of[bs], core_of[bt]
    ks_, kt_ = band_of[bs], band_of[bt]
    js_, jt_ = slot_of[bs], slot_of[bt]
    col_s = offs[ks_] + js_ * capv[ks_] + ws_
    col_t = offs[kt_] + jt_ * capv[kt_] + wt_

    xsb = xsf.astype(BF16)
    xtb = xtf.astype(BF16)

    xs_all = np.zeros((N_CORES, D, ct), dtype=np.float16)
    xt_all = np.zeros((N_CORES, D, ct), dtype=np.float16)
    xs_all[cs_, :, col_s] = xsf.astype(np.float16)
    xt_all[ct_, :, col_t] = xtf.astype(np.float16)
    xp_all = np.zeros((N_CORES, 2 * D, ct), dtype=BF16)
    xp_all[cs_, :D, col_s] = xsb
    xp_all[ct_, D:, col_t] = xtb

    vs_all = np.zeros((N_CORES, cap0, GPC, VW), dtype=BF16)
    vt_all = np.zeros((N_CORES, cap0, GPC, VW), dtype=BF16)
    vs_all[cs_, ws_, 4 * ks_ + js_, :D] = xsb
    vs_all[cs_, ws_, 4 * ks_ + js_, D] = 1.0
    vt_all[ct_, wt_, 4 * kt_ + jt_, :D] = xtb
    vt_all[ct_, wt_, 4 * kt_ + jt_, D] = 1.0

    w1a = np.asarray(w1, dtype=np.float32)
    w2a = np.asarray(w2, dtype=np.float32)
    b1a = np.asarray(b1, dtype=np.float32).reshape(D)
    b2a = np.asarray(b2, dtype=np.float32).reshape(D)
    cwb = np.zeros((2 * D, 6 * D), dtype=np.float32)
    cwb[:D, 0:D] = w1a; cwb[D:, D:2 * D] = w1a
    cwb[:D, 2 * D:3 * D] = w2a; cwb[D:, 3 * D:4 * D] = w2a
    cwb[:, 4 * D:6 * D] = np.eye(2 * D, dtype=np.float32)
    cbf = np.stack([np.concatenate([b1a, b1a]),
                    np.concatenate([b2a, b2a])], axis=1)

    in_maps = []
    for c in range(N_CORES):
        in_maps.append({
            "xs": xs_all[c], "xt": xt_all[c], "xp": xp_all[c],
            "vs": np.ascontiguousarray(vs_all[c].reshape(cap0, GPC * VW)),
            "vt": np.ascontiguousarray(vt_all[c].reshape(cap0, GPC * VW)),
            "cwb": cwb.astype(BF16), "cbf": np.ascontiguousarray(cbf),
        })
    meta = (cs_, ct_, col_s, col_t)
    return in_maps, meta


def _numpy_fallback(x_src, batch_src, x_tar, batch_tar, w1, b1, w2, b2):
    bs = np.asarray(batch_src); bt = np.asarray(batch_tar)
    xs = np.asarray(x_src, dtype=np.float64); xt = np.asarray(x_tar, dtype=np.float64)
    mask = bs[:, None] == bt[None, :]

    def attend(q, kv, m):
        s = np.where(m, q @ kv.T, -1.0e9)
        s = s - s.max(axis=1, keepdims=True)
        e = np.exp(s)
        a = e / e.sum(axis=1, keepdims=True)
        out = a @ kv + q
        return np.where(m.any(axis=1, keepdims=True), out, 0.0)

    def mlp(x):
        return np.maximum(x @ w1 + b1, 0.0) @ w2 + b2 + x

    es = mlp(attend(xs, xt, mask))
    et = mlp(attend(xt, xs, mask.T))
    return et.astype(np.float32), es.astype(np.float32)


def kernel(x_src, batch_src, x_tar, batch_tar, w1, b1, w2, b2):
    bs = np.asarray(batch_src).astype(np.int64)
    bt = np.asarray(batch_tar).astype(np.int64)
    if bs.min(initial=0) < 0 or bs.max(initial=0) >= G \
            or bt.min(initial=0) < 0 or bt.max(initial=0) >= G \
            or not (np.all(np.diff(bs) >= 0) and np.all(np.diff(bt) >= 0)):
        return _numpy_fallback(
            x_src, batch_src, x_tar, batch_tar, w1, b1, w2, b2)
    cnt_s = np.bincount(bs, minlength=G)
    cnt_t = np.bincount(bt, minlength=G)
    if max(cnt_s.max(initial=0), cnt_t.max(initial=0)) > 125:
        return _numpy_fallback(
            x_src, batch_src, x_tar, batch_tar, w1, b1, w2, b2)

    plan = _plan(cnt_s, cnt_t)
    caps = plan[0]
    in_maps, meta = _shard_inputs(
        x_src, batch_src, x_tar, batch_tar, w1, b1, w2, b2, plan)
    cs_, ct_, col_s, col_t = meta

    import os
    from concourse import bass_utils
    if caps not in _PROGRAM_CACHE:
        _PROGRAM_CACHE[caps] = _build_program(caps)
    nc = _PROGRAM_CACHE[caps]
    trace = bool(os.environ.get("KERNEL_TRACE"))
    res = bass_utils.run_bass_kernel_spmd(
        nc, in_maps, core_ids=list(range(N_CORES)), trace=trace)
    _PROGRAM_CACHE["last_result"] = res

    outs = np.stack([np.asarray(res.results[c]["outp"]) for c in range(N_CORES)])
    embed_src = np.ascontiguousarray(outs[cs_, :D, col_s])
    embed_tar = np.ascontiguousarray(outs[ct_, D:, col_t])
    embed_src[cnt_t[bs] == 0] = 0.0
    embed_tar[cnt_s[bt] == 0] = 0.0
    return embed_tar, embed_src


# revision 9
# speedup vs baseline: 2.5689x; 1.2065x over previous
"""Block-diagonal cross-attention + MLP for trn2, 8-core data-parallel.

v4: size-sorted graph banding + fp16 score matmuls + superpair batching.

- 128 graphs sorted by size desc into 4 bands of 32; band k gets a shared
  cap[k] = roundup(max size, 8). Each core takes 4 graphs from each band
  (one superpair/SP of 4 graphs per band) -> identical shapes across cores
  (SPMD) with ~30% less padding than a global cap.
- Scores in fp16 (4x the mantissa of bf16 at the same PE speed); E, values,
  weights, eT in bf16; PSUM/normalization in fp32; output fp32.
- Per SP: 8 score MMs into a 2-bank psum [cap, 1024] (4 blocks/bank, never
  crossing a bank), ONE exp ACT (strided) -> et bf16, 8 V MMs into the
  psum slot (values + mask-col rowsum), ONE reciprocal, ONE broadcast-AP
  normalize mul -> er bf16, 4 PE transposes -> tp psum, ONE fused
  evict+residual add -> eT.
- MLP over eT in <=512 chunks; bias1+relu alternates ACT/DVE; residual via
  identity matmul; per-chunk output DMA.
- 7 input DMAs + 3 output DMAs total, split across sync/gpsimd queues.
"""

from contextlib import ExitStack

import numpy as np
import ml_dtypes

BF16 = ml_dtypes.bfloat16

N_NODES = 8192
D = 64
G = 128
N_CORES = 8
GPC = G // N_CORES          # graphs per core = 16
VW = D + 1                  # value width incl. mask column
NBAND = 4                   # bands == superpairs per core
BANDG = G // NBAND          # graphs per band = 32

_PROGRAM_CACHE = {}


def _build_program(caps):
    import concourse.bass as bass
    import concourse.tile as tile
    from concourse import bacc, mybir

    fp32 = mybir.dt.float32
    bf16 = mybir.dt.bfloat16
    fp16 = mybir.dt.float16
    caps = list(caps)
    cap0 = max(caps)
    off = [4 * sum(caps[:k]) for k in range(NBAND)]
    ct = 4 * sum(caps)                  # total node columns per core
    nc = bacc.Bacc("TRN2", target_bir_lowering=False, debug=False)

    xs = nc.declare_dram_parameter("xs", [D, ct], fp16, isOutput=False)
    xt = nc.declare_dram_parameter("xt", [D, ct], fp16, isOutput=False)
    xp = nc.declare_dram_parameter("xp", [2 * D, ct], bf16, isOutput=False)
    vs = nc.declare_dram_parameter("vs", [cap0, GPC * VW], bf16, isOutput=False)
    vt = nc.declare_dram_parameter("vt", [cap0, GPC * VW], bf16, isOutput=False)
    # w1 | w2 | ident packed [128, 384] bf16; b1 | b2 packed [128, 2] fp32
    cwb = nc.declare_dram_parameter("cwb", [2 * D, 6 * D], bf16, isOutput=False)
    cbf = nc.declare_dram_parameter("cbf", [2 * D, 2], fp32, isOutput=False)
    # chunk-major output: chunk ci lives at rows [128*ci, 128*(ci+1)) so each
    # chunk DMA is one contiguous DRAM block (HWDGE-friendly)
    nch = -(-ct // 512)
    cw = -(-(-(-ct // nch)) // 8) * 8
    outp = nc.declare_dram_parameter("outp", [nch * 2 * D, cw], fp32, isOutput=True)

    AF = mybir.ActivationFunctionType
    ALU = mybir.AluOpType

    with tile.TileContext(nc) as tc, ExitStack() as ctx:
        singles = ctx.enter_context(tc.tile_pool(name="singles", bufs=1))
        epool = ctx.enter_context(tc.tile_pool(name="epool", bufs=3))
        work = ctx.enter_context(tc.tile_pool(name="work", bufs=3))
        opool = ctx.enter_context(tc.tile_pool(name="opool", bufs=3))

        sb_xs = singles.tile([D, ct], fp16, tag="xs")
        sb_xt = singles.tile([D, ct], fp16, tag="xt")
        sb_xp = singles.tile([2 * D, ct], bf16, tag="xp")
        sb_vs = singles.tile([cap0, GPC * VW], bf16, tag="vs")
        sb_vt = singles.tile([cap0, GPC * VW], bf16, tag="vt")
        sb_cw = singles.tile([2 * D, 6 * D], bf16, tag="cwb")
        sb_cb = singles.tile([2 * D, 2], fp32, tag="cbf")
        sb_eT = singles.tile([2 * D, ct], bf16, tag="eT")
        sb_h = singles.tile([2 * D, ct], bf16, tag="h")
        sb_w1 = sb_cw[:, 0:2 * D]
        sb_w2 = sb_cw[:, 2 * D:4 * D]
        sb_id = sb_cw[:, 4 * D:6 * D]
        sb_b1 = sb_cb[:, 0:1]
        sb_b2 = sb_cb[:, 1:2]

        # two HWDGE queues (sync + scalar); x first so compute starts early
        nc.sync.dma_start(out=sb_xs, in_=xs[:, :])
        nc.scalar.dma_start(out=sb_xt, in_=xt[:, :])
        nc.sync.dma_start(out=sb_vs, in_=vs[:, :])
        nc.scalar.dma_start(out=sb_vt, in_=vt[:, :])
        nc.sync.dma_start(out=sb_xp, in_=xp[:, :])
        nc.scalar.dma_start(out=sb_cw, in_=cwb[:, :])
        nc.sync.dma_start(out=sb_cb, in_=cbf[:, :])

        def scol(cap, j, d):
            return 512 * (j // 2) + ((j % 2) * 2 + d) * cap

        def ocol(j, d):
            return 512 * (j // 2) + ((j % 2) * 2 + d) * VW

        scs, ets, ers = {}, {}, {}
        with tc.tile_pool(name="ps_a", bufs=3, space="PSUM") as ps_a, \
             tc.tile_pool(name="ps_t", bufs=2, space="PSUM") as ps_t:
            # skewed pipeline: iter k emits scores_k/exp_k, V_{k-1}, tp_{k-2}
            # so the PE stream never waits on the scalar/vector stages
            for k in range(NBAND + 2):
                if k < NBAND:
                    cap = caps[k]
                    ob = off[k]
                    sc = ps_a.tile([cap, 1024], fp32, tag="sco")
                    scs[k] = sc
                    for j in range(4):
                        xsj = sb_xs[:, ob + j * cap: ob + (j + 1) * cap]
                        xtj = sb_xt[:, ob + j * cap: ob + (j + 1) * cap]
                        nc.tensor.matmul(sc[:, scol(cap, j, 0):scol(cap, j, 0) + cap],
                                         xsj, xtj, start=True, stop=True)
                        nc.tensor.matmul(sc[:, scol(cap, j, 1):scol(cap, j, 1) + cap],
                                         xtj, xsj, start=True, stop=True)
                    et = epool.tile([cap, 8 * cap], bf16, tag="E")
                    ets[k] = et
                    sc_str = sc.rearrange("p (a c) -> p a c", a=2)[:, :, 0:4 * cap]
                    nc.scalar.activation(out=et, in_=sc_str, func=AF.Exp)

                if 1 <= k <= NBAND:
                    kk = k - 1
                    cap = caps[kk]
                    sc, et = scs[kk], ets[kk]
                    # V matmuls reuse the score psum slot (WAR on exp)
                    for j in range(4):
                        vcol = (4 * kk + j) * VW
                        nc.tensor.matmul(
                            sc[:, ocol(j, 0):ocol(j, 0) + VW],
                            et[:, (2 * j + 1) * cap:(2 * j + 2) * cap],
                            sb_vt[0:cap, vcol:vcol + VW], start=True, stop=True)
                        nc.tensor.matmul(
                            sc[:, ocol(j, 1):ocol(j, 1) + VW],
                            et[:, (2 * j) * cap:(2 * j + 1) * cap],
                            sb_vs[0:cap, vcol:vcol + VW], start=True, stop=True)
                    o4 = sc.rearrange("p (a c) -> p a c", a=2)[:, :, 0:4 * VW] \
                           .rearrange("p a (b w) -> p a b w", w=VW)
                    rc = work.tile([cap, 8], fp32, tag="rc")
                    nc.vector.reciprocal(out=rc, in_=o4[:, :, 0:4, D:D + 1])
                    er = work.tile([cap, 8 * D], bf16, tag="er")
                    ers[kk] = er
                    rcb = rc.rearrange("p (a b) -> p a b", a=2).to_broadcast(
                        [cap, 2, 4, D])
                    nc.vector.tensor_mul(er.rearrange("p (a b w) -> p a b w",
                                                      a=2, w=D),
                                         o4[:, :, 0:4, 0:D], rcb)

                if k >= 2:
                    kk = k - 2
                    cap = caps[kk]
                    ob = off[kk]
                    er = ers.pop(kk)
                    tp = ps_t.tile([2 * D, 4 * cap], bf16, tag="tp")
                    for j in range(4):
                        nc.tensor.transpose(tp[:, j * cap:(j + 1) * cap],
                                            er[:, j * 2 * D:(j + 1) * 2 * D],
                                            sb_id[0:cap, 0:cap])
                    nc.vector.tensor_add(sb_eT[:, ob:ob + 4 * cap], tp,
                                         sb_xp[:, ob:ob + 4 * cap])

        # ---- packed MLP over [128, ct] in chunks ----
        bounds = [(i * cw, min(ct, (i + 1) * cw)) for i in range(nch)]
        with tc.tile_pool(name="ps_m", bufs=4, space="PSUM") as ps_m:
            for ci, (c, e) in enumerate(bounds):
                w = e - c
                hp = ps_m.tile([2 * D, cw], fp32, tag="m")
                nc.tensor.matmul(hp[:, 0:w], sb_w1, sb_eT[:, c:c + w],
                                 start=True, stop=True)
                if ci % 2 == 0:
                    nc.scalar.activation(out=sb_h[:, c:c + w], in_=hp[:, 0:w],
                                         func=AF.Relu, bias=sb_b1, scale=1.0)
                else:
                    nc.vector.tensor_scalar(
                        out=sb_h[:, c:c + w], in0=hp[:, 0:w], scalar1=sb_b1,
                        scalar2=0.0, op0=ALU.add, op1=ALU.max)
            for ci, (c, e) in enumerate(bounds):
                w = e - c
                op2 = ps_m.tile([2 * D, cw], fp32, tag="m")
                nc.tensor.matmul(op2[:, 0:w], sb_w2, sb_h[:, c:c + w],
                                 start=True, stop=False)
                nc.tensor.matmul(op2[:, 0:w], sb_id, sb_eT[:, c:c + w],
                                 start=False, stop=True)
                ot = opool.tile([2 * D, cw], fp32, tag="out")
                if ci % 2 == 0:
                    nc.scalar.activation(out=ot[:, 0:w], in_=op2[:, 0:w],
                                         func=AF.Identity, bias=sb_b2, scale=1.0)
                else:
                    nc.vector.tensor_scalar_add(ot[:, 0:w], op2[:, 0:w], sb_b2)
                # full-cw contiguous chunk write (tail cols are junk; host
                # only reads the valid columns)
                nc.sync.dma_start(out=outp[ci * 2 * D:(ci + 1) * 2 * D, :],
                                  in_=ot)

    nc.compile()
    return nc


def _plan(cnt_s, cnt_t):
    size = np.maximum(cnt_s, cnt_t)
    order = np.argsort(-size, kind="stable")
    bands = order.reshape(NBAND, BANDG)
    caps = tuple(int(-(-int(size[b].max()) // 8) * 8) for b in bands)
    core_of = np.empty(G, np.int64)
    band_of = np.empty(G, np.int64)
    slot_of = np.empty(G, np.int64)
    for k in range(NBAND):
        for c in range(N_CORES):
            for j in range(4):
                g = bands[k, c * 4 + j]
                core_of[g] = c
                band_of[g] = k
                slot_of[g] = j
    return caps, core_of, band_of, slot_of


def _shard_inputs(x_src, batch_src, x_tar, batch_tar, w1, b1, w2, b2, plan):
    caps, core_of, band_of, slot_of = plan
    bs = np.asarray(batch_src).astype(np.int64)
    bt = np.asarray(batch_tar).astype(np.int64)
    xsf = np.asarray(x_src, dtype=np.float32)
    xtf = np.asarray(x_tar, dtype=np.float32)
    cap0 = max(caps)
    offs = np.array([4 * sum(caps[:k]) for k in range(NBAND)], np.int64)
    capv = np.array(caps, np.int64)
    ct = int(4 * sum(caps))

    bnd_s = np.searchsorted(bs, np.arange(G + 1))
    bnd_t = np.searchsorted(bt, np.arange(G + 1))
    ws_ = np.arange(N_NODES) - bnd_s[bs]
    wt_ = np.arange(N_NODES) - bnd_t[bt]
    cs_, ct_ = core_of[bs], core_of[bt]
    ks_, kt_ = band_of[bs], band_of[bt]
    js_, jt_ = slot_of[bs], slot_of[bt]
    col_s = offs[ks_] + js_ * capv[ks_] + ws_
    col_t = offs[kt_] + jt_ * capv[kt_] + wt_

    xsb = xsf.astype(BF16)
    xtb = xtf.astype(BF16)

    xs_all = np.zeros((N_CORES, D, ct), dtype=np.float16)
    xt_all = np.zeros((N_CORES, D, ct), dtype=np.float16)
    xs_all[cs_, :, col_s] = xsf.astype(np.float16)
    xt_all[ct_, :, col_t] = xtf.astype(np.float16)
    xp_all = np.zeros((N_CORES, 2 * D, ct), dtype=BF16)
    xp_all[cs_, :D, col_s] = xsb
    xp_all[ct_, D:, col_t] = xtb

    vs_all = np.zeros((N_CORES, cap0, GPC, VW), dtype=BF16)
    vt_all = np.zeros((N_CORES, cap0, GPC, VW), dtype=BF16)
    vs_all[cs_, ws_, 4 * ks_ + js_, :D] = xsb
    vs_all[cs_, ws_, 4 * ks_ + js_, D] = 1.0
    vt_all[ct_, wt_, 4 * kt_ + jt_, :D] = xtb
    vt_all[ct_, wt_, 4 * kt_ + jt_, D] = 1.0

    w1a = np.asarray(w1, dtype=np.float32)
    w2a = np.asarray(w2, dtype=np.float32)
    b1a = np.asarray(b1, dtype=np.float32).reshape(D)
    b2a = np.asarray(b2, dtype=np.float32).reshape(D)
    cwb = np.zeros((2 * D, 6 * D), dtype=np.float32)
    cwb[:D, 0:D] = w1a; cwb[D:, D:2 * D] = w1a
    cwb[:D, 2 * D:3 * D] = w2a; cwb[D:, 3 * D:4 * D] = w2a
    cwb[:, 4 * D:6 * D] = np.eye(2 * D, dtype=np.float32)
    cbf = np.stack([np.concatenate([b1a, b1a]),
                    np.concatenate([b2a, b2a])], axis=1)

    in_maps = []
    for c in range(N_CORES):
        in_maps.append({
            "xs": xs_all[c], "xt": xt_all[c], "xp": xp_all[c],
            "vs": np.ascontiguousarray(vs_all[c].reshape(cap0, GPC * VW)),
            "vt": np.ascontiguousarray(vt_all[c].reshape(cap0, GPC * VW)),
            "cwb": cwb.astype(BF16), "cbf": np.ascontiguousarray(cbf),
        })
    meta = (cs_, ct_, col_s, col_t)
    return in_maps, meta


def _numpy_fallback(x_src, batch_src, x_tar, batch_tar, w1, b1, w2, b2):
    bs = np.asarray(batch_src); bt = np.asarray(batch_tar)
    xs = np.asarray(x_src, dtype=np.float64); xt = np.asarray(x_tar, dtype=np.float64)
    mask = bs[:, None] == bt[None, :]

    def attend(q, kv, m):
        s = np.where(m, q @ kv.T, -1.0e9)
        s = s - s.max(axis=1, keepdims=True)
        e = np.exp(s)
        a = e / e.sum(axis=1, keepdims=True)
        out = a @ kv + q
        return np.where(m.any(axis=1, keepdims=True), out, 0.0)

    def mlp(x):
        return np.maximum(x @ w1 + b1, 0.0) @ w2 + b2 + x

    es = mlp(attend(xs, xt, mask))
    et = mlp(attend(xt, xs, mask.T))
    return et.astype(np.float32), es.astype(np.float32)


def kernel(x_src, batch_src, x_tar, batch_tar, w1, b1, w2, b2):
    bs = np.asarray(batch_src).astype(np.int64)
    bt = np.asarray(batch_tar).astype(np.int64)
    if bs.min(initial=0) < 0 or bs.max(initial=0) >= G \
            or bt.min(initial=0) < 0 or bt.max(initial=0) >= G \
            or not (np.all(np.diff(bs) >= 0) and np.all(np.diff(bt) >= 0)):
        return _numpy_fallback(
            x_src, batch_src, x_tar, batch_tar, w1, b1, w2, b2)
    cnt_s = np.bincount(bs, minlength=G)
    cnt_t = np.bincount(bt, minlength=G)
    if max(cnt_s.max(initial=0), cnt_t.max(initial=0)) > 125:
        return _numpy_fallback(
            x_src, batch_src, x_tar, batch_tar, w1, b1, w2, b2)

    plan = _plan(cnt_s, cnt_t)
    caps = plan[0]
    in_maps, meta = _shard_inputs(
        x_src, batch_src, x_tar, batch_tar, w1, b1, w2, b2, plan)
    cs_, ct_, col_s, col_t = meta

    import os
    from concourse import bass_utils
    if caps not in _PROGRAM_CACHE:
        _PROGRAM_CACHE[caps] = _build_program(caps)
    nc = _PROGRAM_CACHE[caps]
    trace = bool(os.environ.get("KERNEL_TRACE"))
    res = bass_utils.run_bass_kernel_spmd(
        nc, in_maps, core_ids=list(range(N_CORES)), trace=trace)
    _PROGRAM_CACHE["last_result"] = res

    ct_cols = 4 * sum(caps)
    nch = -(-ct_cols // 512)
    cw = -(-(-(-ct_cols // nch)) // 8) * 8
    outs = np.stack([
        np.asarray(res.results[c]["outp"]).reshape(nch, 2 * D, cw)
        .transpose(1, 0, 2).reshape(2 * D, nch * cw)[:, :ct_cols]
        for c in range(N_CORES)])
    embed_src = np.ascontiguousarray(outs[cs_, :D, col_s])
    embed_tar = np.ascontiguousarray(outs[ct_, D:, col_t])
    embed_src[cnt_t[bs] == 0] = 0.0
    embed_tar[cnt_s[bt] == 0] = 0.0
    return embed_tar, embed_src
